# revision 8
# baseline (speedup 1.0000x reference)
"""Trainium2 Bass kernel for nn_Attention_9887014715893.

Multi-head attention forward (B=1, S=4096, D=1024, H=16, E=64, fp32):
    qkv = x @ w_qkv ; q,k,v per head ; softmax(q k^T / 8 + mask) @ v

Sharding: tensor-parallel over heads. 8 cores x 2 heads each. Each core gets
the full x (transposed on host) and its own 128-column slices of w_qkv, and
produces out[:, 128c:128c+128]. No collectives needed.

Per-core algorithm (fp16 matmul data, fp32 PSUM accumulation):
  - x is staged resident in SBUF ([128 d, 8*4096], 64KB/partition), DMA'd
    s-chunk-major so early projection chunks can start after ~1MB arrives.
  - proj: QT2/KT2 [128, 4096] (two heads stacked on the partition axis,
    1/sqrt(E) folded into wq on host). V computed as VT chunks then
    PE-transposed into [s, e] layout augmented with a ones column
    (V_aug [128, 32*65]) so the softmax denominator falls out of the
    attention*V matmul as row 64.
  - emission order overlaps proj with attention: K-proj for all chunks
    first, then Q-proj chunk 0 + V chunks 0-1, then the attention stream
    with the remaining V chunks and Q chunks injected just-in-time — the
    ACT engine (exp, the 2nd-busiest engine) starts ~14us into the kernel
    instead of after the whole projection.
  - attention, scores kept TRANSPOSED (k on partitions, q on free axis):
      scoresT[k_tile, q_chunk] = KT^T-slice x QT-slice   (PE, 2 heads packed
        into row-groups 0-63 / 64-127 of the systolic array)
      expT = exp(scoresT)                                 (ACT, PSUM->SBUF)
      accT[65, q_chunk] += V_aug[k_tile]^T @ expT         (PE, PSUM accum)
    accT rows 0..63 = unnormalized out^T, row 64 = softmax denominator.
    AV matmuls are issued PIPE slots behind their exp: the PE sequencer is
    in-order, so an AV issued right after its scores would head-of-line
    block the next scores matmul on the ACT engine's exp latency
    (measured -60us on hardware).
  - epilogue: DMA the raw [65, q] accumulators to HBM; the divide by the
    denominator row and the final [e, s] -> [s, e] transpose happen on the
    host during the gather (removes 64 PE transposes + DVE scaling from the
    device critical path).
"""

import sys

if "/opt/trn_rl_repo" not in sys.path:
    sys.path.insert(0, "/opt/trn_rl_repo")

import numpy as np
from contextlib import ExitStack

import concourse.bacc as bacc
import concourse.tile as tile
import concourse.mybir as mybir
from concourse.bass_utils import run_bass_kernel_spmd
from concourse.masks import make_identity

F32 = mybir.dt.float32
F16 = mybir.dt.float16
EXP = mybir.ActivationFunctionType.Exp
NP_F16 = np.float16

S = 4096          # sequence length
DM = 1024         # model dim
E = 64            # head dim
NCORES = 8
EC = 128          # output columns per core (2 heads x 64)
NK = S // 128     # 32 k tiles
ND = DM // 128    # 8 d tiles
PIPE = 4          # AV matmul lag (in (ex,head) slots) behind the exp


def _build_kernel(with_mask: bool):
    nc = bacc.Bacc("TRN2", target_bir_lowering=False, debug=False,
                   enable_asserts=False, num_devices=NCORES)
    xT = nc.dram_tensor("xT", [DM, S], F16, kind="ExternalInput").ap()
    wq = nc.dram_tensor("wq", [DM, EC], F16, kind="ExternalInput").ap()
    wk = nc.dram_tensor("wk", [DM, EC], F16, kind="ExternalInput").ap()
    wv = nc.dram_tensor("wv", [DM, EC], F16, kind="ExternalInput").ap()
    if with_mask:
        maskT = nc.dram_tensor("maskT", [S, S], F32, kind="ExternalInput").ap()
    # raw transposed output: rows 0-64 head0 {outT | denom}, 65-129 head1.
    outT = nc.dram_tensor("outT", [130, S], F32, kind="ExternalOutput").ap()

    with tile.TileContext(nc) as tc, ExitStack() as ctx:
        const_pool = ctx.enter_context(tc.tile_pool(name="const", bufs=1))
        ident_f = const_pool.tile([128, 128], F32)
        make_identity(nc, ident_f)
        ident = const_pool.tile([128, 128], F16)
        nc.vector.tensor_copy(ident[:], ident_f[:])

        w_pool = ctx.enter_context(tc.tile_pool(name="w", bufs=1))
        wq_sb = w_pool.tile([128, DM], F16)
        wk_sb = w_pool.tile([128, DM], F16)
        wv_sb = w_pool.tile([128, DM], F16)
        for t in range(ND):
            nc.sync.dma_start(wq_sb[:, 128 * t:128 * (t + 1)], wq[128 * t:128 * (t + 1), :])
            nc.sync.dma_start(wk_sb[:, 128 * t:128 * (t + 1)], wk[128 * t:128 * (t + 1), :])
            nc.sync.dma_start(wv_sb[:, 128 * t:128 * (t + 1)], wv[128 * t:128 * (t + 1), :])

        # x resident in SBUF: [128 d-part, ND*S] (d-tile t -> cols [t*S,(t+1)*S))
        x_pool = ctx.enter_context(tc.tile_pool(name="xf", bufs=1))
        xsf = x_pool.tile([128, ND * S], F16)
        # s-chunk-major load order so chunk-0 projections start early
        for c in range(ND):
            for t in range(ND):
                nc.sync.dma_start(
                    xsf[:, t * S + 512 * c: t * S + 512 * (c + 1)],
                    xT[128 * t:128 * (t + 1), 512 * c:512 * (c + 1)])

        qt_pool = ctx.enter_context(tc.tile_pool(name="qt", bufs=1))
        QT2 = qt_pool.tile([128, S], F16)   # rows 0-63 head0 e-dims, 64-127 head1
        KT2 = qt_pool.tile([128, S], F16)
        va_pool = ctx.enter_context(tc.tile_pool(name="va", bufs=1))
        va = [va_pool.tile([128, 65 * NK], F16, name=f"va{h}") for h in range(2)]
        ones_f = const_pool.tile([128, 1], F32)
        nc.vector.memset(ones_f[:], 1.0)
        for h in range(2):
            nc.vector.tensor_copy(va[h][:, 64:65 * NK:65],
                                  ones_f[:].to_broadcast([128, NK]))

        vt_pool = ctx.enter_context(tc.tile_pool(name="vt", bufs=2))
        # psA: proj psums, V-transpose psums and transposed-scores psums
        psA = ctx.enter_context(tc.tile_pool(name="psA", bufs=3, space="PSUM"))
        # psB: the two per-head AV accumulators (1 bank each)
        psB = ctx.enter_context(tc.tile_pool(name="psB", bufs=2, space="PSUM"))
        exp_pool = ctx.enter_context(tc.tile_pool(name="exp", bufs=8))
        accsb_pool = ctx.enter_context(tc.tile_pool(name="accsb", bufs=4))
        if with_mask:
            msk_pool = ctx.enter_context(tc.tile_pool(name="msk", bufs=3))

        def proj(dst, wsb, c):
            ps = psA.tile([128, 512], F32, tag="psA")
            for t in range(ND):
                nc.tensor.matmul(
                    ps[:], lhsT=wsb[:, 128 * t:128 * (t + 1)],
                    rhs=xsf[:, t * S + 512 * c: t * S + 512 * (c + 1)],
                    start=(t == 0), stop=(t == ND - 1))
            nc.vector.tensor_copy(dst[:, 512 * c:512 * (c + 1)], ps[:])

        def vproj(c):
            ps = psA.tile([128, 512], F32, tag="psA")
            for t in range(ND):
                nc.tensor.matmul(
                    ps[:], lhsT=wv_sb[:, 128 * t:128 * (t + 1)],
                    rhs=xsf[:, t * S + 512 * c: t * S + 512 * (c + 1)],
                    start=(t == 0), stop=(t == ND - 1))
            vts = vt_pool.tile([128, 512], F16, tag="vt")
            nc.vector.tensor_copy(vts[:], ps[:])
            for st in range(4):
                kk = 4 * c + st
                tp = psA.tile([128, 128], F16, tag="psA")
                nc.tensor.transpose(tp[:], vts[:, 128 * st:128 * (st + 1)],
                                    ident[:])
                nc.vector.tensor_copy(va[0][:, 65 * kk:65 * kk + 64], tp[:, 0:64])
                nc.vector.tensor_copy(va[1][:, 65 * kk:65 * kk + 64], tp[:, 64:128])

        # K first (attention needs all of KT2), then just enough Q/V to start
        for c in range(ND):
            proj(KT2, wk_sb, c)
        proj(QT2, wq_sb, 0)
        vproj(0)
        vproj(1)

        # ---------------- attention ----------------
        for qc in range(S // 512):
            q0 = 512 * qc
            accs = [psB.tile([65, 512], F32, tag="psB", name=f"acc{qc}_{h}")
                    for h in range(2)]
            av_queue = []  # pending (ex, kp, h)

            def issue_av(ex, kp_, h_):
                for cc in range(2):
                    kk = 2 * kp_ + cc
                    nc.tensor.matmul(
                        accs[h_][:],
                        lhsT=va[h_][:, 65 * kk:65 * kk + 65],
                        rhs=ex[:, 512 * cc:512 * (cc + 1)],
                        start=(kk == 0), stop=(kk == NK - 1),
                    )

            for kp in range(NK // 2):
                # just-in-time projection injections (qc 0 consumes V chunk
                # c at kp=2c..2c+1; 2-kp lead). Q chunk qc+1 mid-loop.
                if qc == 0 and kp % 2 == 0 and 2 <= kp <= 12:
                    vproj(kp // 2 + 1)
                if qc == 0 and kp == 14:
                    proj(QT2, wq_sb, 1)
                if 1 <= qc <= 6 and kp == 8:
                    proj(QT2, wq_sb, qc + 1)
                k0 = 256 * kp
                if with_mask:
                    msk = msk_pool.tile([128, 1024], F32, tag="msk")
                    nc.sync.dma_start(msk[:, 0:512], maskT[k0:k0 + 128, q0:q0 + 512])
                    nc.sync.dma_start(msk[:, 512:1024],
                                      maskT[k0 + 128:k0 + 256, q0:q0 + 512])
                for h in range(2):
                    sc_ps = psA.tile([128, 1024], F32, tag="psA", name=f"sc{kp}_{h}")
                    for cc in range(2):
                        nc.tensor.matmul(
                            sc_ps[:, 512 * cc:512 * (cc + 1)],
                            lhsT=KT2[64 * h:64 * (h + 1), k0 + 128 * cc:k0 + 128 * (cc + 1)],
                            rhs=QT2[64 * h:64 * (h + 1), q0:q0 + 512],
                            start=True, stop=True,
                            tile_position=(64 * h, 0),
                        )
                    if with_mask:
                        nc.vector.tensor_tensor(out=sc_ps[:], in0=sc_ps[:],
                                                in1=msk[:], op=mybir.AluOpType.add)
                    ex = exp_pool.tile([128, 1024], F16, tag="exp", name=f"ex{kp}_{h}")
                    nc.scalar.activation(ex[:], sc_ps[:], EXP)
                    av_queue.append((ex, kp, h))
                    while len(av_queue) > PIPE:
                        issue_av(*av_queue.pop(0))
            while av_queue:
                issue_av(*av_queue.pop(0))
            # epilogue for this q chunk: evacuate accs to SBUF, DMA out raw
            for h in range(2):
                asb = accsb_pool.tile([65, 512], F32, tag="accsb")
                nc.vector.tensor_copy(asb[:], accs[h][:])
                nc.sync.dma_start(outT[65 * h:65 * h + 65, q0:q0 + 512], asb[:])

    nc.compile()
    return nc


_CACHE: dict = {}


def _get_kernel(with_mask: bool):
    if with_mask not in _CACHE:
        _CACHE[with_mask] = _build_kernel(with_mask)
    return _CACHE[with_mask]


def make_in_maps(x: np.ndarray, w_qkv: np.ndarray, maskT=None):
    xT = np.ascontiguousarray(x[0].T).astype(NP_F16)      # [DM, S]
    scale = np.float32(1.0 / np.sqrt(E))
    in_maps = []
    for c in range(NCORES):
        m = {
            "xT": xT,
            "wq": (np.ascontiguousarray(w_qkv[:, EC * c:EC * (c + 1)]) * scale
                   ).astype(NP_F16),
            "wk": np.ascontiguousarray(
                w_qkv[:, DM + EC * c:DM + EC * (c + 1)]).astype(NP_F16),
            "wv": np.ascontiguousarray(
                w_qkv[:, 2 * DM + EC * c:2 * DM + EC * (c + 1)]).astype(NP_F16),
        }
        if maskT is not None:
            m["maskT"] = maskT
        in_maps.append(m)
    return in_maps


def kernel(x: np.ndarray, mask: np.ndarray, w_qkv: np.ndarray) -> np.ndarray:
    x = np.asarray(x, dtype=np.float32)
    mask = np.asarray(mask, dtype=np.float32)
    w_qkv = np.asarray(w_qkv, dtype=np.float32)
    assert x.shape == (1, S, DM) and w_qkv.shape == (DM, 3 * DM)

    with_mask = bool(np.any(mask))
    nc = _get_kernel(with_mask)

    maskT = None
    if with_mask:
        maskT = np.ascontiguousarray(np.broadcast_to(mask, (1, 1, S, S))[0, 0].T)
    in_maps = make_in_maps(x, w_qkv, maskT)

    res = run_bass_kernel_spmd(nc, in_maps, core_ids=list(range(NCORES)))
    # host-side normalize (softmax denominator is row 64/129) and transpose
    outs = []
    for c in range(NCORES):
        o = res.results[c]["outT"]                       # [130, S]
        h0 = o[0:64] / o[64:65]
        h1 = o[65:129] / o[129:130]
        outs.append(np.concatenate([h0, h1], axis=0).T)  # [S, 128]
    return np.ascontiguousarray(
        np.concatenate(outs, axis=1), dtype=np.float32).reshape(1, S, DM)


# revision 9
# speedup vs baseline: 1.1028x; 1.1028x over previous
"""Trainium2 Bass kernel for nn_Attention_9887014715893.

Multi-head attention forward (B=1, S=4096, D=1024, H=16, E=64, fp32):
    qkv = x @ w_qkv ; q,k,v per head ; softmax(q k^T / 8 + mask) @ v

Sharding: tensor-parallel over heads. 8 cores x 2 heads each. Each core gets
the full x (transposed on host) and its own 128-column slices of w_qkv, and
produces out[:, 128c:128c+128]. No collectives needed.

Per-core algorithm (fp16 matmul data, fp32 PSUM accumulation):
  - x is staged resident in SBUF ([128 d, 8*4096], 64KB/partition), DMA'd
    s-chunk-major so early projection chunks can start after ~1MB arrives.
  - proj: QT2/KT2 [128, 4096] (two heads stacked on the partition axis,
    1/sqrt(E) folded into wq on host). V computed as VT chunks then
    PE-transposed into [s, e] layout augmented with a ones column
    (V_aug [128, 32*65]) so the softmax denominator falls out of the
    attention*V matmul as row 64.
  - emission order overlaps proj with attention: K-proj for all chunks
    first, then Q-proj chunk 0 + V chunks 0-1, then the attention stream
    with the remaining V chunks and Q chunks injected just-in-time — the
    ACT engine (exp, the 2nd-busiest engine) starts ~14us into the kernel
    instead of after the whole projection.
  - attention, scores kept TRANSPOSED (k on partitions, q on free axis):
      scoresT[k_tile, q_chunk] = KT^T-slice x QT-slice   (PE, 2 heads packed
        into row-groups 0-63 / 64-127 of the systolic array)
      expT = exp(scoresT)                                 (ACT, PSUM->SBUF)
      accT[65, q_chunk] += V_aug[k_tile]^T @ expT         (PE, PSUM accum)
    accT rows 0..63 = unnormalized out^T, row 64 = softmax denominator.
    AV matmuls are issued PIPE slots behind their exp: the PE sequencer is
    in-order, so an AV issued right after its scores would head-of-line
    block the next scores matmul on the ACT engine's exp latency
    (measured -60us on hardware).
  - epilogue: DMA the raw [65, q] accumulators to HBM; the divide by the
    denominator row and the final [e, s] -> [s, e] transpose happen on the
    host during the gather (removes 64 PE transposes + DVE scaling from the
    device critical path).
"""

import sys

if "/opt/trn_rl_repo" not in sys.path:
    sys.path.insert(0, "/opt/trn_rl_repo")

import numpy as np
from contextlib import ExitStack

import concourse.bacc as bacc
import concourse.tile as tile
import concourse.mybir as mybir
from concourse.bass_utils import run_bass_kernel_spmd
from concourse.masks import make_identity

F32 = mybir.dt.float32
F16 = mybir.dt.float16
EXP = mybir.ActivationFunctionType.Exp
NP_F16 = np.float16

S = 4096          # sequence length
DM = 1024         # model dim
E = 64            # head dim
NCORES = 8
EC = 128          # output columns per core (2 heads x 64)
NK = S // 128     # 32 k tiles
ND = DM // 128    # 8 d tiles
PIPE = 4          # AV matmul lag (in (ex,head) slots) behind the exp


def _build_kernel(with_mask: bool):
    nc = bacc.Bacc("TRN2", target_bir_lowering=False, debug=False,
                   enable_asserts=False, num_devices=NCORES)
    xT = nc.dram_tensor("xT", [DM, S], F16, kind="ExternalInput").ap()
    # single fused weight input ([wq*scale | wk | wv] columns) -> fewer jit
    # args, lower per-call dispatch cost in chained timing loops
    wqkv = nc.dram_tensor("wqkv", [DM, 3 * EC], F16, kind="ExternalInput").ap()
    if with_mask:
        maskT = nc.dram_tensor("maskT", [S, S], F32, kind="ExternalInput").ap()
    # raw transposed output: rows 0-64 head0 {outT | denom}, 65-129 head1.
    outT = nc.dram_tensor("outT", [130, S], F32, kind="ExternalOutput").ap()

    with tile.TileContext(nc) as tc, ExitStack() as ctx:
        const_pool = ctx.enter_context(tc.tile_pool(name="const", bufs=1))
        ident_f = const_pool.tile([128, 128], F32)
        make_identity(nc, ident_f)
        ident = const_pool.tile([128, 128], F16)
        nc.vector.tensor_copy(ident[:], ident_f[:])

        w_pool = ctx.enter_context(tc.tile_pool(name="w", bufs=1))
        wq_sb = w_pool.tile([128, DM], F16)
        wk_sb = w_pool.tile([128, DM], F16)
        wv_sb = w_pool.tile([128, DM], F16)
        for t in range(ND):
            nc.sync.dma_start(wq_sb[:, 128 * t:128 * (t + 1)],
                              wqkv[128 * t:128 * (t + 1), 0:EC])
            nc.sync.dma_start(wk_sb[:, 128 * t:128 * (t + 1)],
                              wqkv[128 * t:128 * (t + 1), EC:2 * EC])
            nc.sync.dma_start(wv_sb[:, 128 * t:128 * (t + 1)],
                              wqkv[128 * t:128 * (t + 1), 2 * EC:3 * EC])

        # x resident in SBUF: [128 d-part, ND*S] (d-tile t -> cols [t*S,(t+1)*S))
        x_pool = ctx.enter_context(tc.tile_pool(name="xf", bufs=1))
        xsf = x_pool.tile([128, ND * S], F16)
        # s-chunk-major load order so chunk-0 projections start early
        for c in range(ND):
            for t in range(ND):
                nc.sync.dma_start(
                    xsf[:, t * S + 512 * c: t * S + 512 * (c + 1)],
                    xT[128 * t:128 * (t + 1), 512 * c:512 * (c + 1)])

        qt_pool = ctx.enter_context(tc.tile_pool(name="qt", bufs=1))
        QT2 = qt_pool.tile([128, S], F16)   # rows 0-63 head0 e-dims, 64-127 head1
        KT2 = qt_pool.tile([128, S], F16)
        va_pool = ctx.enter_context(tc.tile_pool(name="va", bufs=1))
        va = [va_pool.tile([128, 65 * NK], F16, name=f"va{h}") for h in range(2)]
        ones_f = const_pool.tile([128, 1], F32)
        nc.vector.memset(ones_f[:], 1.0)
        for h in range(2):
            nc.vector.tensor_copy(va[h][:, 64:65 * NK:65],
                                  ones_f[:].to_broadcast([128, NK]))

        vt_pool = ctx.enter_context(tc.tile_pool(name="vt", bufs=2))
        # psA: proj psums, V-transpose psums and transposed-scores psums
        psA = ctx.enter_context(tc.tile_pool(name="psA", bufs=3, space="PSUM"))
        # psB: the two per-head AV accumulators (1 bank each)
        psB = ctx.enter_context(tc.tile_pool(name="psB", bufs=2, space="PSUM"))
        exp_pool = ctx.enter_context(tc.tile_pool(name="exp", bufs=8))
        accsb_pool = ctx.enter_context(tc.tile_pool(name="accsb", bufs=4))
        if with_mask:
            msk_pool = ctx.enter_context(tc.tile_pool(name="msk", bufs=3))

        def proj(dst, wsb, c):
            ps = psA.tile([128, 512], F32, tag="psA")
            for t in range(ND):
                nc.tensor.matmul(
                    ps[:], lhsT=wsb[:, 128 * t:128 * (t + 1)],
                    rhs=xsf[:, t * S + 512 * c: t * S + 512 * (c + 1)],
                    start=(t == 0), stop=(t == ND - 1))
            nc.vector.tensor_copy(dst[:, 512 * c:512 * (c + 1)], ps[:])

        def vproj(c):
            ps = psA.tile([128, 512], F32, tag="psA")
            for t in range(ND):
                nc.tensor.matmul(
                    ps[:], lhsT=wv_sb[:, 128 * t:128 * (t + 1)],
                    rhs=xsf[:, t * S + 512 * c: t * S + 512 * (c + 1)],
                    start=(t == 0), stop=(t == ND - 1))
            vts = vt_pool.tile([128, 512], F16, tag="vt")
            nc.vector.tensor_copy(vts[:], ps[:])
            for st in range(4):
                kk = 4 * c + st
                tp = psA.tile([128, 128], F16, tag="psA")
                nc.tensor.transpose(tp[:], vts[:, 128 * st:128 * (st + 1)],
                                    ident[:])
                nc.vector.tensor_copy(va[0][:, 65 * kk:65 * kk + 64], tp[:, 0:64])
                nc.vector.tensor_copy(va[1][:, 65 * kk:65 * kk + 64], tp[:, 64:128])

        # K first (attention needs all of KT2), then just enough Q/V to start
        for c in range(ND):
            proj(KT2, wk_sb, c)
        proj(QT2, wq_sb, 0)
        vproj(0)
        vproj(1)

        # ---------------- attention ----------------
        for qc in range(S // 512):
            q0 = 512 * qc
            accs = [psB.tile([65, 512], F32, tag="psB", name=f"acc{qc}_{h}")
                    for h in range(2)]
            av_queue = []  # pending (ex, kp, h)

            def issue_av(ex, kp_, h_):
                for cc in range(2):
                    kk = 2 * kp_ + cc
                    nc.tensor.matmul(
                        accs[h_][:],
                        lhsT=va[h_][:, 65 * kk:65 * kk + 65],
                        rhs=ex[:, 512 * cc:512 * (cc + 1)],
                        start=(kk == 0), stop=(kk == NK - 1),
                    )

            for kp in range(NK // 2):
                # just-in-time projection injections (qc 0 consumes V chunk
                # c at kp=2c..2c+1; 2-kp lead). Q chunk qc+1 mid-loop.
                if qc == 0 and kp % 2 == 0 and 2 <= kp <= 12:
                    vproj(kp // 2 + 1)
                if qc == 0 and kp == 14:
                    proj(QT2, wq_sb, 1)
                if 1 <= qc <= 6 and kp == 8:
                    proj(QT2, wq_sb, qc + 1)
                k0 = 256 * kp
                if with_mask:
                    msk = msk_pool.tile([128, 1024], F32, tag="msk")
                    nc.sync.dma_start(msk[:, 0:512], maskT[k0:k0 + 128, q0:q0 + 512])
                    nc.sync.dma_start(msk[:, 512:1024],
                                      maskT[k0 + 128:k0 + 256, q0:q0 + 512])
                for h in range(2):
                    sc_ps = psA.tile([128, 1024], F32, tag="psA", name=f"sc{kp}_{h}")
                    for cc in range(2):
                        nc.tensor.matmul(
                            sc_ps[:, 512 * cc:512 * (cc + 1)],
                            lhsT=KT2[64 * h:64 * (h + 1), k0 + 128 * cc:k0 + 128 * (cc + 1)],
                            rhs=QT2[64 * h:64 * (h + 1), q0:q0 + 512],
                            start=True, stop=True,
                            tile_position=(64 * h, 0),
                        )
                    if with_mask:
                        nc.vector.tensor_tensor(out=sc_ps[:], in0=sc_ps[:],
                                                in1=msk[:], op=mybir.AluOpType.add)
                    ex = exp_pool.tile([128, 1024], F16, tag="exp", name=f"ex{kp}_{h}")
                    nc.scalar.activation(ex[:], sc_ps[:], EXP)
                    av_queue.append((ex, kp, h))
                    while len(av_queue) > PIPE:
                        issue_av(*av_queue.pop(0))
            while av_queue:
                issue_av(*av_queue.pop(0))
            # epilogue for this q chunk: evacuate accs to SBUF, DMA out raw
            for h in range(2):
                asb = accsb_pool.tile([65, 512], F32, tag="accsb")
                nc.vector.tensor_copy(asb[:], accs[h][:])
                nc.sync.dma_start(outT[65 * h:65 * h + 65, q0:q0 + 512], asb[:])

    nc.compile()
    return nc


_CACHE: dict = {}


def _get_kernel(with_mask: bool):
    if with_mask not in _CACHE:
        _CACHE[with_mask] = _build_kernel(with_mask)
    return _CACHE[with_mask]


def make_in_maps(x: np.ndarray, w_qkv: np.ndarray, maskT=None):
    xT = np.ascontiguousarray(x[0].T).astype(NP_F16)      # [DM, S]
    scale = np.float32(1.0 / np.sqrt(E))
    in_maps = []
    for c in range(NCORES):
        wqkv = np.concatenate([
            np.ascontiguousarray(w_qkv[:, EC * c:EC * (c + 1)]) * scale,
            w_qkv[:, DM + EC * c:DM + EC * (c + 1)],
            w_qkv[:, 2 * DM + EC * c:2 * DM + EC * (c + 1)],
        ], axis=1).astype(NP_F16)
        m = {"xT": xT, "wqkv": wqkv}
        if maskT is not None:
            m["maskT"] = maskT
        in_maps.append(m)
    return in_maps


def kernel(x: np.ndarray, mask: np.ndarray, w_qkv: np.ndarray) -> np.ndarray:
    x = np.asarray(x, dtype=np.float32)
    mask = np.asarray(mask, dtype=np.float32)
    w_qkv = np.asarray(w_qkv, dtype=np.float32)
    assert x.shape == (1, S, DM) and w_qkv.shape == (DM, 3 * DM)

    with_mask = bool(np.any(mask))
    nc = _get_kernel(with_mask)

    maskT = None
    if with_mask:
        maskT = np.ascontiguousarray(np.broadcast_to(mask, (1, 1, S, S))[0, 0].T)
    in_maps = make_in_maps(x, w_qkv, maskT)

    res = run_bass_kernel_spmd(nc, in_maps, core_ids=list(range(NCORES)))
    # host-side normalize (softmax denominator is row 64/129) and transpose
    outs = []
    for c in range(NCORES):
        o = res.results[c]["outT"]                       # [130, S]
        h0 = o[0:64] / o[64:65]
        h1 = o[65:129] / o[129:130]
        outs.append(np.concatenate([h0, h1], axis=0).T)  # [S, 128]
    return np.ascontiguousarray(
        np.concatenate(outs, axis=1), dtype=np.float32).reshape(1, S, DM)


# revision 10
# speedup vs baseline: 1.3298x; 1.2059x over previous
"""Trainium2 Bass kernel for nn_Attention_9887014715893.

Multi-head attention forward (B=1, S=4096, D=1024, H=16, E=64, fp32):
    qkv = x @ w_qkv ; q,k,v per head ; softmax(q k^T / 8 + mask) @ v

Sharding: tensor-parallel over heads. 8 cores x 2 heads each. Each core gets
the full x (transposed on host) and its own 128-column slices of w_qkv, and
produces out[:, 128c:128c+128]. No collectives needed.

Per-core algorithm (fp16 matmul data, fp32 PSUM accumulation):
  - x is staged resident in SBUF ([128 d, 8*4096], 64KB/partition), DMA'd
    s-chunk-major so early projection chunks can start after ~1MB arrives.
  - proj: QT2/KT2 [128, 4096] (two heads stacked on the partition axis,
    1/sqrt(E) folded into wq on host). V computed as VT chunks then
    PE-transposed into [s, e] layout augmented with a ones column
    (V_aug [128, 32*65]) so the softmax denominator falls out of the
    attention*V matmul as row 64.
  - emission order overlaps proj with attention: K-proj for all chunks
    first, then Q-proj chunk 0 + V chunks 0-1, then the attention stream
    with the remaining V chunks and Q chunks injected just-in-time — the
    ACT engine (exp, the 2nd-busiest engine) starts ~14us into the kernel
    instead of after the whole projection.
  - attention, scores kept TRANSPOSED (k on partitions, q on free axis):
      scoresT[k_tile, q_chunk] = KT^T-slice x QT-slice   (PE, 2 heads packed
        into row-groups 0-63 / 64-127 of the systolic array)
      expT = exp(scoresT)                                 (ACT, PSUM->SBUF)
      accT[65, q_chunk] += V_aug[k_tile]^T @ expT         (PE, PSUM accum)
    accT rows 0..63 = unnormalized out^T, row 64 = softmax denominator.
    AV matmuls are issued PIPE slots behind their exp: the PE sequencer is
    in-order, so an AV issued right after its scores would head-of-line
    block the next scores matmul on the ACT engine's exp latency
    (measured -60us on hardware).
  - epilogue: DMA the raw [65, q] accumulators to HBM; the divide by the
    denominator row and the final [e, s] -> [s, e] transpose happen on the
    host during the gather (removes 64 PE transposes + DVE scaling from the
    device critical path).
"""

import sys

if "/opt/trn_rl_repo" not in sys.path:
    sys.path.insert(0, "/opt/trn_rl_repo")

import numpy as np
from contextlib import ExitStack

import concourse.bacc as bacc
import concourse.tile as tile
import concourse.mybir as mybir
from concourse.bass_utils import run_bass_kernel_spmd
from concourse.masks import make_identity

F32 = mybir.dt.float32
F16 = mybir.dt.float16
EXP = mybir.ActivationFunctionType.Exp
NP_F16 = np.float16

S = 4096          # sequence length
DM = 1024         # model dim
E = 64            # head dim
NCORES = 8
EC = 128          # output columns per core (2 heads x 64)
NK = S // 128     # 32 k tiles
ND = DM // 128    # 8 d tiles
PIPE = 4          # AV matmul lag (in (ex,head) slots) behind the exp


def _build_kernel(with_mask: bool):
    nc = bacc.Bacc("TRN2", target_bir_lowering=False, debug=False,
                   enable_asserts=False, num_devices=NCORES)
    xT = nc.dram_tensor("xT", [DM, S], F16, kind="ExternalInput").ap()
    # single fused weight input ([wq*scale | wk | wv] columns) -> fewer jit
    # args, lower per-call dispatch cost in chained timing loops
    wqkv = nc.dram_tensor("wqkv", [DM, 3 * EC], F16, kind="ExternalInput").ap()
    if with_mask:
        maskT = nc.dram_tensor("maskT", [S, S], F32, kind="ExternalInput").ap()
    # raw transposed output: rows 0-64 head0 {outT | denom}, 65-129 head1.
    outT = nc.dram_tensor("outT", [130, S], F32, kind="ExternalOutput").ap()

    with tile.TileContext(nc) as tc, ExitStack() as ctx:
        const_pool = ctx.enter_context(tc.tile_pool(name="const", bufs=1))
        ident_f = const_pool.tile([128, 128], F32)
        make_identity(nc, ident_f)
        ident = const_pool.tile([128, 128], F16)
        nc.vector.tensor_copy(ident[:], ident_f[:])

        w_pool = ctx.enter_context(tc.tile_pool(name="w", bufs=1))
        wq_sb = w_pool.tile([128, DM], F16)
        wk_sb = w_pool.tile([128, DM], F16)
        wv_sb = w_pool.tile([128, DM], F16)
        for t in range(ND):
            nc.sync.dma_start(wq_sb[:, 128 * t:128 * (t + 1)],
                              wqkv[128 * t:128 * (t + 1), 0:EC])
            nc.sync.dma_start(wk_sb[:, 128 * t:128 * (t + 1)],
                              wqkv[128 * t:128 * (t + 1), EC:2 * EC])
            nc.sync.dma_start(wv_sb[:, 128 * t:128 * (t + 1)],
                              wqkv[128 * t:128 * (t + 1), 2 * EC:3 * EC])

        # x resident in SBUF: [128 d-part, ND*S] (d-tile t -> cols [t*S,(t+1)*S))
        x_pool = ctx.enter_context(tc.tile_pool(name="xf", bufs=1))
        xsf = x_pool.tile([128, ND * S], F16)
        # s-chunk-major load order so chunk-0 projections start early
        for c in range(ND):
            for t in range(ND):
                nc.sync.dma_start(
                    xsf[:, t * S + 512 * c: t * S + 512 * (c + 1)],
                    xT[128 * t:128 * (t + 1), 512 * c:512 * (c + 1)])

        qt_pool = ctx.enter_context(tc.tile_pool(name="qt", bufs=1))
        QT2 = qt_pool.tile([128, S], F16)   # rows 0-63 head0 e-dims, 64-127 head1
        KT2 = qt_pool.tile([128, S], F16)
        va_pool = ctx.enter_context(tc.tile_pool(name="va", bufs=1))
        va = [va_pool.tile([128, 65 * NK], F16, name=f"va{h}") for h in range(2)]
        ones_f = const_pool.tile([128, 1], F32)
        nc.vector.memset(ones_f[:], 1.0)
        for h in range(2):
            nc.vector.tensor_copy(va[h][:, 64:65 * NK:65],
                                  ones_f[:].to_broadcast([128, NK]))

        vt_pool = ctx.enter_context(tc.tile_pool(name="vt", bufs=2))
        # psA: proj psums, V-transpose psums and transposed-scores psums
        psA = ctx.enter_context(tc.tile_pool(name="psA", bufs=3, space="PSUM"))
        # psB: the two per-head AV accumulators (1 bank each)
        psB = ctx.enter_context(tc.tile_pool(name="psB", bufs=2, space="PSUM"))
        exp_pool = ctx.enter_context(tc.tile_pool(name="exp", bufs=8))
        accsb_pool = ctx.enter_context(tc.tile_pool(name="accsb", bufs=4))
        if with_mask:
            msk_pool = ctx.enter_context(tc.tile_pool(name="msk", bufs=3))

        def proj(dst, wsb, c):
            ps = psA.tile([128, 512], F32, tag="psA")
            for t in range(ND):
                nc.tensor.matmul(
                    ps[:], lhsT=wsb[:, 128 * t:128 * (t + 1)],
                    rhs=xsf[:, t * S + 512 * c: t * S + 512 * (c + 1)],
                    start=(t == 0), stop=(t == ND - 1))
            nc.vector.tensor_copy(dst[:, 512 * c:512 * (c + 1)], ps[:])

        def vproj(c):
            ps = psA.tile([128, 512], F32, tag="psA")
            for t in range(ND):
                nc.tensor.matmul(
                    ps[:], lhsT=wv_sb[:, 128 * t:128 * (t + 1)],
                    rhs=xsf[:, t * S + 512 * c: t * S + 512 * (c + 1)],
                    start=(t == 0), stop=(t == ND - 1))
            vts = vt_pool.tile([128, 512], F16, tag="vt")
            nc.vector.tensor_copy(vts[:], ps[:])
            for st in range(4):
                kk = 4 * c + st
                tp = psA.tile([128, 128], F16, tag="psA")
                nc.tensor.transpose(tp[:], vts[:, 128 * st:128 * (st + 1)],
                                    ident[:])
                nc.vector.tensor_copy(va[0][:, 65 * kk:65 * kk + 64], tp[:, 0:64])
                nc.vector.tensor_copy(va[1][:, 65 * kk:65 * kk + 64], tp[:, 64:128])

        # Just enough proj to start attention: K chunks 0-1, Q chunk 0,
        # V chunks 0-1. K chunks 2-7 are injected into the attention stream
        # (scores at kp consume K chunk kp//2, injected 2-3 kp ahead).
        proj(KT2, wk_sb, 0)
        proj(KT2, wk_sb, 1)
        proj(QT2, wq_sb, 0)
        vproj(0)
        vproj(1)

        # ---------------- attention ----------------
        for qc in range(S // 512):
            q0 = 512 * qc
            accs = [psB.tile([65, 512], F32, tag="psB", name=f"acc{qc}_{h}")
                    for h in range(2)]
            av_queue = []  # pending (ex, kp, h)

            def issue_av(ex, kp_, h_):
                for cc in range(2):
                    kk = 2 * kp_ + cc
                    nc.tensor.matmul(
                        accs[h_][:],
                        lhsT=va[h_][:, 65 * kk:65 * kk + 65],
                        rhs=ex[:, 512 * cc:512 * (cc + 1)],
                        start=(kk == 0), stop=(kk == NK - 1),
                    )

            for kp in range(NK // 2):
                # just-in-time projection injections (qc 0 consumes V chunk
                # c at kp=2c..2c+1; 2-kp lead). Q chunk qc+1 mid-loop.
                if qc == 0 and kp % 2 == 0 and 2 <= kp <= 12:
                    vproj(kp // 2 + 1)
                if qc == 0 and kp % 2 == 1 and kp <= 11:
                    proj(KT2, wk_sb, (kp + 3) // 2)   # K chunks 2..7
                if qc == 0 and kp == 14:
                    proj(QT2, wq_sb, 1)
                if 1 <= qc <= 6 and kp == 8:
                    proj(QT2, wq_sb, qc + 1)
                k0 = 256 * kp
                if with_mask:
                    msk = msk_pool.tile([128, 1024], F32, tag="msk")
                    nc.sync.dma_start(msk[:, 0:512], maskT[k0:k0 + 128, q0:q0 + 512])
                    nc.sync.dma_start(msk[:, 512:1024],
                                      maskT[k0 + 128:k0 + 256, q0:q0 + 512])
                for h in range(2):
                    sc_ps = psA.tile([128, 1024], F32, tag="psA", name=f"sc{kp}_{h}")
                    for cc in range(2):
                        nc.tensor.matmul(
                            sc_ps[:, 512 * cc:512 * (cc + 1)],
                            lhsT=KT2[64 * h:64 * (h + 1), k0 + 128 * cc:k0 + 128 * (cc + 1)],
                            rhs=QT2[64 * h:64 * (h + 1), q0:q0 + 512],
                            start=True, stop=True,
                            tile_position=(64 * h, 0),
                        )
                    if with_mask:
                        nc.vector.tensor_tensor(out=sc_ps[:], in0=sc_ps[:],
                                                in1=msk[:], op=mybir.AluOpType.add)
                    ex = exp_pool.tile([128, 1024], F16, tag="exp", name=f"ex{kp}_{h}")
                    nc.scalar.activation(ex[:], sc_ps[:], EXP)
                    av_queue.append((ex, kp, h))
                    while len(av_queue) > PIPE:
                        issue_av(*av_queue.pop(0))
            while av_queue:
                issue_av(*av_queue.pop(0))
            # epilogue for this q chunk: evacuate accs to SBUF, DMA out raw
            for h in range(2):
                asb = accsb_pool.tile([65, 512], F32, tag="accsb")
                nc.vector.tensor_copy(asb[:], accs[h][:])
                nc.sync.dma_start(outT[65 * h:65 * h + 65, q0:q0 + 512], asb[:])

    nc.compile()
    return nc


_CACHE: dict = {}


def _get_kernel(with_mask: bool):
    if with_mask not in _CACHE:
        _CACHE[with_mask] = _build_kernel(with_mask)
    return _CACHE[with_mask]


def make_in_maps(x: np.ndarray, w_qkv: np.ndarray, maskT=None):
    xT = np.ascontiguousarray(x[0].T).astype(NP_F16)      # [DM, S]
    scale = np.float32(1.0 / np.sqrt(E))
    in_maps = []
    for c in range(NCORES):
        wqkv = np.concatenate([
            np.ascontiguousarray(w_qkv[:, EC * c:EC * (c + 1)]) * scale,
            w_qkv[:, DM + EC * c:DM + EC * (c + 1)],
            w_qkv[:, 2 * DM + EC * c:2 * DM + EC * (c + 1)],
        ], axis=1).astype(NP_F16)
        m = {"xT": xT, "wqkv": wqkv}
        if maskT is not None:
            m["maskT"] = maskT
        in_maps.append(m)
    return in_maps


def kernel(x: np.ndarray, mask: np.ndarray, w_qkv: np.ndarray) -> np.ndarray:
    x = np.asarray(x, dtype=np.float32)
    mask = np.asarray(mask, dtype=np.float32)
    w_qkv = np.asarray(w_qkv, dtype=np.float32)
    assert x.shape == (1, S, DM) and w_qkv.shape == (DM, 3 * DM)

    with_mask = bool(np.any(mask))
    nc = _get_kernel(with_mask)

    maskT = None
    if with_mask:
        maskT = np.ascontiguousarray(np.broadcast_to(mask, (1, 1, S, S))[0, 0].T)
    in_maps = make_in_maps(x, w_qkv, maskT)

    res = run_bass_kernel_spmd(nc, in_maps, core_ids=list(range(NCORES)))
    # host-side normalize (softmax denominator is row 64/129) and transpose
    outs = []
    for c in range(NCORES):
        o = res.results[c]["outT"]                       # [130, S]
        h0 = o[0:64] / o[64:65]
        h1 = o[65:129] / o[129:130]
        outs.append(np.concatenate([h0, h1], axis=0).T)  # [S, 128]
    return np.ascontiguousarray(
        np.concatenate(outs, axis=1), dtype=np.float32).reshape(1, S, DM)
